# revision 2
# baseline (speedup 1.0000x reference)
"""Trainium2 Bass kernel for nn_BitInput: exact-count random bitstream sampler.

For each scalar probability p in x[256,1024], emits a 256-bit stream with
exactly round(p*256) ones at uniformly-random positions (matches the
reference distribution exactly; RNG stream is our own, as sampling allows).

Algorithm:
  host:   n_ones = round-half-even(x*256); split each element's count over
          32 interleaved blocks of 8 positions via an exact multivariate-
          hypergeometric binary tree (numpy Generator.hypergeometric).
  device: per block, 8-step Fisher-Yates sequential sampling
          bit_s = [u * (8-s)/65536 < k_rem], k_rem -= bit_s
          with u = (gpsimd xorwow) XOR (per-partition host mask, read with a
          per-step shifted offset). Endpoints are exact for any u, so the
          per-element counts are exactly n_ones.
  Output bits staged in fp16, cast to f32 by the SWDGE DMA on the way out.

Sharding: data parallel over 8 NeuronCores, 32768 elements each.
"""
import os
import sys

import numpy as np

for _p in ("/opt/trn_rl_repo", "/root/.axon_site/_ro/trn_rl_repo"):
    if os.path.isdir(_p) and _p not in sys.path:
        sys.path.append(_p)

import concourse.bass as bass  # noqa: E402
import concourse.mybir as mybir  # noqa: E402
from concourse import bacc  # noqa: E402
from concourse.tile import TileContext  # noqa: E402
from concourse.bass_utils import run_bass_kernel_spmd  # noqa: E402

P = 128  # SBUF partitions
C = 32  # blocks per element (bit position = s*C + c)
B = 8  # block length = FY steps
NBITS = C * B  # 256
EPC = 256  # elements per partition (per core): 32768 / 128
NCORES = 8
EF = 64  # elements-per-partition per chunk
NCHUNK = EPC // EF
FD = EF * C  # free dim per step instruction

_cache = {}


def _build(n_devices=NCORES):
    nc = bacc.Bacc(
        "TRN2", target_bir_lowering=False, debug=False, num_devices=n_devices
    )
    k_in = nc.dram_tensor("k_in", [P, EPC * C], mybir.dt.float16, kind="ExternalInput")
    r_in = nc.dram_tensor(
        "r_in", [P, EPC * C + B], mybir.dt.uint16, kind="ExternalInput"
    )
    out = nc.dram_tensor(
        "out", [P * EPC, NBITS], mybir.dt.float32, kind="ExternalOutput"
    )
    out3 = out.ap().rearrange("(p f) b -> p f b", p=P)

    with TileContext(nc) as tc:
        with (
            tc.tile_pool(name="rmask", bufs=1) as rpool,
            tc.tile_pool(name="state", bufs=2) as spool,
            tc.tile_pool(name="randg", bufs=2) as ugpool,
            tc.tile_pool(name="rande", bufs=2) as uepool,
            tc.tile_pool(name="stage", bufs=2) as gpool,
        ):
            r_sb = rpool.tile([P, EPC * C + B], mybir.dt.uint16)
            nc.sync.dma_start(out=r_sb[:], in_=r_in[:])

            def gen_randoms():
                ugs = []
                for s in range(B - 1):
                    ug = ugpool.tile([P, EF, C], mybir.dt.uint16, tag=f"ug{s}")
                    nc.gpsimd.random(ug[:])
                    ugs.append(ug)
                return ugs

            # unique per-step ug tags with bufs=2: the xorwow fills for chunk
            # q+1 are issued while chunk q computes, and a slot is only
            # rewritten after its consumer two chunks back has long finished.
            ugs_cur = gen_randoms()
            for q in range(NCHUNK):
                ugs_next = gen_randoms() if q + 1 < NCHUNK else None
                k_sb = spool.tile([P, EF, C], mybir.dt.float16, tag="k")
                nc.sync.dma_start(
                    out=k_sb[:],
                    in_=k_in[:, q * FD : (q + 1) * FD].rearrange(
                        "p (e c) -> p e c", c=C
                    ),
                )
                stg = gpool.tile([P, EF, B, C], mybir.dt.float16, tag="stg")
                ue = None
                for s in range(B):
                    if s < B - 1:
                        ue = uepool.tile([P, EF, C], mybir.dt.uint16, tag="ue")
                        rmask = r_sb[:, q * FD + s : q * FD + s + FD].rearrange(
                            "p (e c) -> p e c", c=C
                        )
                        nc.vector.tensor_tensor(
                            out=ue[:],
                            in0=ugs_cur[s][:],
                            in1=rmask,
                            op=mybir.AluOpType.bitwise_xor,
                        )
                    # bit = (u * (B-s)/65536) < k_rem  (exact at k_rem=0 / =n_rem)
                    nc.vector.scalar_tensor_tensor(
                        out=stg[:, :, s, :],
                        in0=ue[:],
                        scalar=float(B - s) / 65536.0,
                        in1=k_sb[:],
                        op0=mybir.AluOpType.mult,
                        op1=mybir.AluOpType.is_lt,
                    )
                    if s < B - 1:
                        nc.vector.tensor_tensor(
                            out=k_sb[:],
                            in0=k_sb[:],
                            in1=stg[:, :, s, :],
                            op=mybir.AluOpType.subtract,
                        )
                nc.gpsimd.dma_start(
                    out=out3[:, q * EF : (q + 1) * EF, :],
                    in_=stg[:].rearrange("p e s c -> p e (s c)"),
                )
                ugs_cur = ugs_next
    nc.finalize()
    return nc


def _tree_split(k, rng):
    """Exact multivariate hypergeometric: split counts (over 256 positions)
    into 32 leaf counts (8 positions each) via nested binary splits."""
    counts = k[:, None].astype(np.int64)
    m = NBITS
    while m > B:
        half = m // 2
        left = rng.hypergeometric(counts, m - counts, half)
        counts = np.stack([left, counts - left], axis=-1).reshape(counts.shape[0], -1)
        m = half
    return counts  # [n, 32], each in 0..8, rows sum to k


def _prepare(x):
    rng = np.random.default_rng(0xB17B17)
    n = x.size
    perm = rng.permutation(n)  # device slot -> original element
    k = np.rint(x.reshape(-1).astype(np.float32) * np.float32(NBITS)).astype(np.int64)
    k = np.clip(k, 0, NBITS)
    leaves = _tree_split(k[perm], rng)  # [n, 32]
    k_in = leaves.reshape(NCORES, P, EPC, C).astype(np.float16)
    r = rng.integers(0, 65536, size=(NCORES, P, EPC * C + B), dtype=np.uint16)
    in_maps = [
        {"k_in": k_in[ci].reshape(P, EPC * C), "r_in": r[ci]} for ci in range(NCORES)
    ]
    return in_maps, perm, k


def kernel(x, bit_size):
    x = np.asarray(x)
    assert int(bit_size) == NBITS, f"kernel hardcoded for bit_size=256, got {bit_size}"
    assert x.shape == (256, 1024), x.shape
    in_maps, perm, k = _prepare(x)
    if "nc" not in _cache:
        _cache["nc"] = _build()
    res = run_bass_kernel_spmd(
        _cache["nc"], in_maps, core_ids=list(range(NCORES)), trace=_cache.get("trace", False)
    )
    _cache["last_result"] = res
    outs = np.concatenate([r["out"] for r in res.results], axis=0)  # [262144, 256]
    result = np.empty((x.size, NBITS), dtype=np.float32)
    result[perm] = outs
    return result.reshape(256, 1024, NBITS).astype(x.dtype, copy=False)


# revision 11
# speedup vs baseline: 1.2940x; 1.2940x over previous
"""Trainium2 Bass kernel for nn_BitInput: exact-count random bitstream sampler.

For each scalar probability p in x[256,1024], emits a 256-bit stream with
exactly round(p*256) ones at uniformly-random positions (matches the
reference distribution exactly; RNG stream is our own, as sampling allows).

Algorithm:
  host:   n_ones = round-half-even(x*256); split each element's count over
          32 interleaved blocks of 8 positions via an exact multivariate-
          hypergeometric binary tree (numpy Generator.hypergeometric).
  device: per block, 8-step Fisher-Yates sequential sampling
          bit_s = [u * (8-s)/65536 < k_rem], k_rem -= bit_s
          with u = (gpsimd xorwow) XOR (per-partition host mask, read with a
          per-step shifted offset). Endpoints are exact for any u, so the
          per-element counts are exactly n_ones.
  Output bits staged in fp16, cast to f32 by the SWDGE DMA on the way out.

Sharding: data parallel over 8 NeuronCores, 32768 elements each.
"""
import os
import sys

import numpy as np

for _p in ("/opt/trn_rl_repo", "/root/.axon_site/_ro/trn_rl_repo"):
    if os.path.isdir(_p) and _p not in sys.path:
        sys.path.append(_p)

import concourse.bass as bass  # noqa: E402
import concourse.mybir as mybir  # noqa: E402
from concourse import bacc  # noqa: E402
from concourse.tile import TileContext  # noqa: E402
from concourse.bass_utils import run_bass_kernel_spmd  # noqa: E402

P = 128  # SBUF partitions
C = 32  # blocks per element (bit position = s*C + c)
B = 8  # block length = FY steps
NBITS = C * B  # 256
EPC = 256  # elements per partition (per core): 32768 / 128
NCORES = 8
EF = 64  # largest elements-per-partition chunk
FD = EF * C  # largest free dim per step instruction
# chunk plan: small first chunk (short pipeline ramp) and small last chunk
# (short drain tail), large chunks in the middle (lower DVE instr overhead).
CHUNKS = [(0, 32), (32, 64), (96, 64), (160, 64), (224, 32)]

_cache = {}


def _build(n_devices=NCORES):
    nc = bacc.Bacc(
        "TRN2", target_bir_lowering=False, debug=False, num_devices=n_devices
    )
    k_in = nc.dram_tensor("k_in", [P, EPC * C], mybir.dt.float16, kind="ExternalInput")
    r_in = nc.dram_tensor("r_in", [P, EPC * C + B], mybir.dt.uint16, kind="ExternalInput")
    out = nc.dram_tensor(
        "out", [P * EPC, NBITS], mybir.dt.float32, kind="ExternalOutput"
    )
    out3 = out.ap().rearrange("(p f) b -> p f b", p=P)

    with TileContext(nc) as tc:
        with (
            tc.tile_pool(name="rmask", bufs=1) as rpool,
            tc.tile_pool(name="state", bufs=3) as spool,
            tc.tile_pool(name="randg", bufs=2) as ugpool,
            tc.tile_pool(name="rande", bufs=3) as uepool,
            tc.tile_pool(name="thresh", bufs=3) as tpool,
            tc.tile_pool(name="stage", bufs=3) as gpool,
        ):
            r_sb = rpool.tile([P, EPC * C + B], mybir.dt.uint16)
            nc.sync.dma_start(out=r_sb[:], in_=r_in[:])

            def gen_randoms(ef):
                ugs = []
                for s in range(B - 1):
                    ug = ugpool.tile([P, ef, C], mybir.dt.uint16, tag=f"ug{s}")
                    nc.gpsimd.random(ug[:])
                    ugs.append(ug)
                return ugs

            # unique per-step ug tags with bufs=2: the xorwow fills for chunk
            # q+1 are issued while chunk q computes, and a slot is only
            # rewritten after its consumer two chunks back has long finished.
            ugs_cur = gen_randoms(CHUNKS[0][1])
            for qi, (e0, ef) in enumerate(CHUNKS):
                fd = ef * C
                ugs_next = (
                    gen_randoms(CHUNKS[qi + 1][1]) if qi + 1 < len(CHUNKS) else None
                )
                k_sb = spool.tile([P, ef, C], mybir.dt.float16, tag="k")
                nc.sync.dma_start(
                    out=k_sb[:],
                    in_=k_in[:, e0 * C : (e0 + ef) * C].rearrange(
                        "p (e c) -> p e c", c=C
                    ),
                )
                stg = gpool.tile([P, ef, B, C], mybir.dt.float16, tag="stg")
                for s in range(B - 1):
                    ue = uepool.tile([P, ef, C], mybir.dt.uint16, tag="ue")
                    rmask = r_sb[:, e0 * C + s : e0 * C + s + fd].rearrange(
                        "p (e c) -> p e c", c=C
                    )
                    nc.vector.tensor_tensor(
                        out=ue[:],
                        in0=ugs_cur[s][:],
                        in1=rmask,
                        op=mybir.AluOpType.bitwise_xor,
                    )
                    # bit = (u * (B-s)/65536) < k_rem  (exact at endpoints)
                    nc.vector.scalar_tensor_tensor(
                        out=stg[:, :, s, :],
                        in0=ue[:],
                        scalar=float(B - s) / 65536.0,
                        in1=k_sb[:],
                        op0=mybir.AluOpType.mult,
                        op1=mybir.AluOpType.is_lt,
                    )
                    nc.vector.tensor_tensor(
                        out=k_sb[:],
                        in0=k_sb[:],
                        in1=stg[:, :, s, :],
                        op=mybir.AluOpType.subtract,
                    )
                # last step: n_rem=1 so bit == k_rem (0 or 1): plain copy (4x)
                nc.vector.tensor_copy(out=stg[:, :, B - 1, :], in_=k_sb[:])
                nc.gpsimd.dma_start(
                    out=out3[:, e0 : e0 + ef, :],
                    in_=stg[:].rearrange("p e s c -> p e (s c)"),
                )
                ugs_cur = ugs_next
    nc.finalize()
    return nc


def _tree_split(k, rng):
    """Exact multivariate hypergeometric: split counts (over 256 positions)
    into 32 leaf counts (8 positions each) via nested binary splits."""
    counts = k[:, None].astype(np.int64)
    m = NBITS
    while m > B:
        half = m // 2
        left = rng.hypergeometric(counts, m - counts, half)
        counts = np.stack([left, counts - left], axis=-1).reshape(counts.shape[0], -1)
        m = half
    return counts  # [n, 32], each in 0..8, rows sum to k


def _prepare(x):
    rng = np.random.default_rng(0xB17B17)
    n = x.size
    perm = rng.permutation(n)  # device slot -> original element
    k = np.rint(x.reshape(-1).astype(np.float32) * np.float32(NBITS)).astype(np.int64)
    k = np.clip(k, 0, NBITS)
    leaves = _tree_split(k[perm], rng)  # [n, 32]
    k_in = leaves.reshape(NCORES, P, EPC, C).astype(np.float16)
    r = rng.integers(0, 65536, size=(NCORES, P, EPC * C + B), dtype=np.uint16)
    in_maps = [
        {"k_in": k_in[ci].reshape(P, EPC * C), "r_in": r[ci]} for ci in range(NCORES)
    ]
    return in_maps, perm, k


def kernel(x, bit_size):
    x = np.asarray(x)
    assert int(bit_size) == NBITS, f"kernel hardcoded for bit_size=256, got {bit_size}"
    assert x.shape == (256, 1024), x.shape
    in_maps, perm, k = _prepare(x)
    if "nc" not in _cache:
        _cache["nc"] = _build()
    res = run_bass_kernel_spmd(
        _cache["nc"], in_maps, core_ids=list(range(NCORES)), trace=_cache.get("trace", False)
    )
    _cache["last_result"] = res
    outs = np.concatenate([r["out"] for r in res.results], axis=0)  # [262144, 256]
    result = np.empty((x.size, NBITS), dtype=np.float32)
    result[perm] = outs
    return result.reshape(256, 1024, NBITS).astype(x.dtype, copy=False)
